# revision 9
# baseline (speedup 1.0000x reference)
"""Trainium2 Bass kernel: multi-head attention (B=4, S=1024, H=16, dk=dv=64, D=1024)
returning (attn [B,H,S,S], out [B,S,D]) like the reference nn.Module.

Sharding: 8 cores = 4 batches x 2 query-halves (pure data parallel, no collectives).
Each core handles (batch b, 512 query rows) against all 1024 keys:
  - QKV projections on the PE in float32r (full-rate fp32) with host-pre-transposed
    operands so the contraction dim (model dim) lands on SBUF partitions.
  - scores^T per head in [k, q] layout; exp on ScalarE with the 1/sqrt(dk) scale
    folded into the activation's scale field; 0/1 mask applied as a multiply
    (masked lanes become exactly 0, matching the reference's -1e9 + softmax).
  - ctx^T = vh_aug.T @ p with an extra ones-column on vh, so the softmax row-sums
    fall out of the same matmul (PSUM row dk).
  - normalize p and ctx by 1/sum (gpsimd partition-broadcast of the reciprocal).
  - O-projection with residual folded in as identity-matrix matmuls on the PE,
    then layernorm (bn_stats/bn_aggr + Sqrt + reciprocal).
Device emits attn^T [H, Sk, Sq_shard]; the host transposes back and reassembles.
"""

import sys
from contextlib import ExitStack

import numpy as np

try:
    import concourse.bass as bass
except ImportError:  # grading env fallback
    sys.path.insert(0, "/opt/trn_rl_repo")
    import concourse.bass as bass

import concourse.tile as tile
from concourse import bacc, mybir
from concourse._compat import with_exitstack
from concourse.bass_utils import run_bass_kernel_spmd

F32 = mybir.dt.float32
F32R = mybir.dt.float32r
AF = mybir.ActivationFunctionType
ALU = mybir.AluOpType
P = 128

# full-problem dims
B, S, DMODEL, H, DK = 4, 1024, 1024, 16, 64
NCORES = 8
SQ = S // 2  # queries per core
LN_EPS = 1e-6


@with_exitstack
def _mha_kernel(ctx: ExitStack, tc, outs, ins, dm=DMODEL, sq=SQ, sk=S, nheads=H):
    nc = tc.nc
    dk = DK
    nt = dm // P  # model-dim 128-tiles
    kt = sk // P  # key-seq 128-tiles
    qm = sq // P  # query 128-tiles
    nf = min(512, dm)  # free-dim chunk for matmul N (PSUM bank limit)
    assert nheads * dk == dm and sq <= 512

    qT, kT, vT, mT = ins["qT"], ins["kT"], ins["vT"], ins["mT"]
    WqT, WkT, WvT, WoT = ins["WqT"], ins["WkT"], ins["WvT"], ins["WoT"]
    gamma, beta = ins["gamma"], ins["beta"]
    attnT_o, out_o = outs["attnT"], outs["out"]

    persist = ctx.enter_context(tc.tile_pool(name="persist", bufs=1))
    psp = ctx.enter_context(tc.tile_pool(name="psp", bufs=4, space="PSUM"))
    psc = ctx.enter_context(tc.tile_pool(name="psc", bufs=2, space="PSUM"))
    pso = ctx.enter_context(tc.tile_pool(name="pso", bufs=1, space="PSUM"))

    # ---- persistent tiles
    qT_sb = persist.tile([P, nt, sq], F32R, tag="qT")  # [d, q] (also residual src)
    nc.sync.dma_start(out=qT_sb, in_=qT.rearrange("(t p) s -> p t s", p=P))
    qhT_sb = persist.tile([P, nt, sq], F32R, tag="qhT")  # [hd, q]
    khT_sb = persist.tile([P, nt, sk], F32R, tag="khT")  # [hd, k]
    # vh with a ones column per head: [k, kt, h, dv+4]; col dk == 1.0
    vh_sb = persist.tile([P, kt, nheads, dk + 4], F32R, tag="vh")
    ctxT_sb = persist.tile([P, nt, sq], F32R, tag="ctxT")  # [hdv, q]
    nc.sync.dma_start(
        out=vh_sb[:, :, :, dk],
        in_=ins["ones"].rearrange("p (t h) -> p t h", h=nheads),
    )

    # ---- phase 1: projections (stream kT/vT and weights in halves to fit SBUF)
    with (
        tc.tile_pool(name="xstream", bufs=2) as xp,
        tc.tile_pool(name="wstream", bufs=2) as wp,
    ):
        def w_half(w_dram, n0, width):
            w_sb = wp.tile([P, nt, width], F32R, tag="w")
            nc.sync.dma_start(
                out=w_sb,
                in_=w_dram.rearrange("(t p) n -> p t n", p=P)[:, :, n0 : n0 + width],
            )
            return w_sb

        # q-projection: qhT[m-tile, :] = sum_d WqT[d, m].T @ qT[d, :]
        for half in range(max(1, (nheads * dk) // nf)):
            w_sb = w_half(WqT, half * nf, min(nf, dm))
            for mm in range(half * nf // P, (half * nf + min(nf, dm)) // P):
                pt = psp.tile([P, sq], F32, tag="ps")
                loc = mm * P - half * nf
                for kk in range(nt):
                    nc.tensor.matmul(
                        pt,
                        (w_sb[:, kk, loc : loc + P]),
                        (qT_sb[:, kk, :]),
                        start=(kk == 0),
                        stop=(kk == nt - 1),
                    )
                nc.any.tensor_copy(out=qhT_sb[:, mm, :], in_=pt)

        # k-projection: khT[m-tile, n-chunk]; kT streamed in sk/nf column chunks
        for n0 in range(0, sk, nf):
            kT_sb = xp.tile([P, nt, nf], F32R, tag="xT")
            nc.sync.dma_start(
                out=kT_sb,
                in_=kT.rearrange("(t p) s -> p t s", p=P)[:, :, n0 : n0 + nf],
            )
            for half in range(max(1, (nheads * dk) // nf)):
                w_sb = w_half(WkT, half * nf, min(nf, dm))
                for mm in range(half * nf // P, (half * nf + min(nf, dm)) // P):
                    pt = psp.tile([P, nf], F32, tag="ps")
                    loc = mm * P - half * nf
                    for kk in range(nt):
                        nc.tensor.matmul(
                            pt,
                            (w_sb[:, kk, loc : loc + P]),
                            (kT_sb[:, kk, :]),
                            start=(kk == 0),
                            stop=(kk == nt - 1),
                        )
                    nc.any.tensor_copy(out=khT_sb[:, mm, n0 : n0 + nf], in_=pt)

        # v-projection: vh[k-tile m, hd-chunk] = sum_d vT[d, mP:+P].T @ WvT[d, chunk]
        for mm in range(kt):
            vT_sb = xp.tile([P, nt, P], F32R, tag="vTs")
            nc.sync.dma_start(
                out=vT_sb,
                in_=vT.rearrange("(t p) s -> p t s", p=P)[:, :, mm * P : (mm + 1) * P],
            )
            for n0 in range(0, nheads * dk, nf):
                w_sb = w_half(WvT, n0, min(nf, dm))
                pt = psp.tile([P, min(nf, dm)], F32, tag="ps")
                for kk in range(nt):
                    nc.tensor.matmul(
                        pt,
                        (vT_sb[:, kk, :]),
                        (w_sb[:, kk, :]),
                        start=(kk == 0),
                        stop=(kk == nt - 1),
                    )
                h0 = n0 // dk
                nhc = min(nf, dm) // dk
                nc.any.tensor_copy(
                    out=vh_sb[:, mm, h0 : h0 + nhc, 0:dk],
                    in_=pt.rearrange("p (h d) -> p h d", d=dk),
                )

    # ---- phase 2: attention per head (wo pool opened early so WoT prefetches)
    wop = ctx.enter_context(tc.tile_pool(name="wo", bufs=2))
    with (
        tc.tile_pool(name="att", bufs=2) as att,
        tc.tile_pool(name="attm", bufs=1) as attm,
        tc.tile_pool(name="atts", bufs=3) as atts,
    ):
        m_sb = attm.tile([P, kt, sq], F32, tag="m")  # mask^T as 0.0/1.0
        nc.sync.dma_start(out=m_sb, in_=mT.rearrange("(t p) s -> p t s", p=P))

        for h in range(nheads):
            j, r0 = h // 2, (h % 2) * dk
            p_sb = att.tile([P, kt, sq], F32R, tag="p")
            for t in range(kt):
                st = psp.tile([P, sq], F32, tag="ps")
                nc.tensor.matmul(
                    st,
                    (khT_sb[r0 : r0 + dk, j, t * P : (t + 1) * P]),
                    (qhT_sb[r0 : r0 + dk, j, :]),
                    start=True,
                    stop=True,
                )
                # p = exp(s / sqrt(dk)); the scale rides the activation for free
                nc.scalar.activation(
                    out=p_sb[:, t, :], in_=st, func=AF.Exp,
                    scale=1.0 / float(np.sqrt(dk)),
                )
                nc.vector.tensor_mul(p_sb[:, t, :], p_sb[:, t, :], m_sb[:, t, :])
            # ctx^T (+ row dk = softmax sums via the ones column)
            ct = psc.tile([P, sq], F32, tag="ps_c")
            for t in range(kt):
                nc.tensor.matmul(
                    ct[0 : dk + 1, :],
                    (vh_sb[:, t, h, 0 : dk + 1]),
                    (p_sb[:, t, :]),
                    start=(t == 0),
                    stop=(t == kt - 1),
                )
            rec = atts.tile([1, sq], F32, tag="rec")
            nc.vector.reciprocal(out=rec, in_=ct[dk : dk + 1, :])
            rb = atts.tile([P, sq], F32, tag="rb")
            nc.gpsimd.partition_broadcast(rb, rec[0:1, :])
            # normalized ctx^T directly into the O-projection operand layout
            nc.vector.tensor_mul(ctxT_sb[r0 : r0 + dk, j, :], ct[0:dk, :], rb[0:dk, :])
            # normalize attention in place and stream out
            rb_b = bass.AP(
                tensor=rb.tensor, offset=rb.offset, ap=[rb.ap[0], [0, kt], rb.ap[1]]
            )
            nc.vector.tensor_mul(p_sb, p_sb, rb_b)
            nc.sync.dma_start(
                out=attnT_o.rearrange("h (t p) q -> h p t q", p=P)[h], in_=p_sb
            )

    # ---- phase 3: O-projection + residual (identity matmuls) + layernorm
    with (
        tc.tile_pool(name="ln", bufs=2) as lnp,
        tc.tile_pool(name="lns", bufs=3) as lns,
        tc.tile_pool(name="singles", bufs=1) as singles,
    ):
        idb = singles.tile([P, nf - P + nf], F32R, tag="idb")
        nc.sync.dma_start(out=idb, in_=ins["idb"])
        gb = singles.tile([P, dm], F32, tag="gb")
        nc.sync.dma_start(
            out=gb,
            in_=bass.AP(tensor=gamma.tensor, offset=gamma.offset, ap=[[0, P], [1, dm]]),
        )
        bb = singles.tile([P, dm], F32, tag="bb")
        nc.sync.dma_start(
            out=bb,
            in_=bass.AP(tensor=beta.tensor, offset=beta.offset, ap=[[0, P], [1, dm]]),
        )
        epst = singles.tile([P, 1], F32, tag="eps")
        nc.vector.memset(epst, LN_EPS)

        for mm in range(qm):
            xt = pso.tile([P, dm], F32, tag="ps_o")
            for n0 in range(0, dm, nf):
                w_sb = wop.tile([P, nt, nf], F32R, tag="wo")
                nc.sync.dma_start(
                    out=w_sb,
                    in_=WoT.rearrange("(t p) n -> p t n", p=P)[:, :, n0 : n0 + nf],
                )
                for j in range(nt):
                    nc.tensor.matmul(
                        xt[:, n0 : n0 + nf],
                        (ctxT_sb[:, j, mm * P : (mm + 1) * P]),
                        (w_sb[:, j, :]),
                        start=(j == 0),
                        stop=False,
                    )
                # residual: out += q via identity-matrix matmuls
                ndt = nf // P
                for dd in range(ndt):
                    kk = (n0 // P) + dd
                    nc.tensor.matmul(
                        xt[:, n0 : n0 + nf],
                        (qT_sb[:, kk, mm * P : (mm + 1) * P]),
                        (idb[:, nf - P - dd * P : 2 * nf - P - dd * P]),
                        start=False,
                        stop=(dd == ndt - 1),
                    )
            # layernorm over the model dim
            nsub = max(1, dm // 512)
            stats = lns.tile([P, nsub, 6], F32, tag="stats")
            for sg in range(nsub):
                w = dm // nsub
                nc.vector.bn_stats(out=stats[:, sg, :], in_=xt[:, sg * w : (sg + 1) * w])
            mv = lns.tile([P, 2], F32, tag="mv")
            nc.vector.bn_aggr(out=mv, in_=stats)
            rstd = lns.tile([P, 1], F32, tag="rstd")
            nc.scalar.activation(out=rstd, in_=mv[:, 1:2], func=AF.Sqrt, bias=epst)
            rstd2 = lns.tile([P, 1], F32, tag="rstd2")
            nc.vector.reciprocal(out=rstd2, in_=rstd)
            nmu = lns.tile([P, 1], F32, tag="nmu")
            nc.vector.tensor_scalar(
                out=nmu, in0=mv[:, 0:1], scalar1=-1.0, scalar2=None, op0=ALU.mult
            )
            yt = lnp.tile([P, dm], F32, tag="y")
            nc.vector.tensor_scalar(
                out=yt, in0=xt, scalar1=nmu, scalar2=rstd2, op0=ALU.add, op1=ALU.mult
            )
            ot = lnp.tile([P, dm], F32, tag="ot")
            nc.vector.tensor_mul(ot, yt, gb)
            nc.vector.tensor_add(ot, ot, bb)
            nc.sync.dma_start(out=out_o.rearrange("(m p) d -> m p d", p=P)[mm], in_=ot)


_CACHE = {}


def build_nc(dm=DMODEL, sq=SQ, sk=S, nheads=H):
    key = (dm, sq, sk, nheads)
    if key in _CACHE:
        return _CACHE[key]
    nc = bacc.Bacc("TRN2", target_bir_lowering=False, debug=False, num_devices=NCORES)
    ins = {}

    def din(name, shape, dt=F32):
        ins[name] = nc.dram_tensor(name, list(shape), dt, kind="ExternalInput").ap()

    nf = min(512, dm)
    din("qT", (dm, sq), F32R)
    din("kT", (dm, sk), F32R)
    din("vT", (dm, sk), F32R)
    din("mT", (sk, sq))
    din("WqT", (dm, nheads * DK), F32R)
    din("WkT", (dm, nheads * DK), F32R)
    din("WvT", (dm, nheads * DK), F32R)
    din("WoT", (nheads * DK, dm), F32R)
    din("gamma", (dm,))
    din("beta", (dm,))
    din("idb", (P, 2 * nf - P), F32R)
    din("ones", (P, (sk // P) * nheads), F32R)
    outs = {
        "attnT": nc.dram_tensor("attnT", [nheads, sk, sq], F32R, kind="ExternalOutput").ap(),
        "out": nc.dram_tensor("out", [sq, dm], F32, kind="ExternalOutput").ap(),
    }
    with tile.TileContext(nc) as tc:
        _mha_kernel(tc, outs, ins, dm=dm, sq=sq, sk=sk, nheads=nheads)
    nc.compile()
    _CACHE[key] = nc
    return nc


def aux_inputs(dm=DMODEL, sk=S, nheads=H):
    """Host-fed constants: identity block (residual matmuls) + all-ones column."""
    nf = min(512, dm)
    idb = np.zeros((P, 2 * nf - P), np.float32)
    idb[np.arange(P), (nf - P) + np.arange(P)] = 1.0
    ones = np.ones((P, (sk // P) * nheads), np.float32)
    return idb, ones


def _core_inputs(q, k, v, mask, WqT, WkT, WvT, WoT, gamma, beta, c):
    b, hf = divmod(c, 2)
    sl = slice(hf * SQ, (hf + 1) * SQ)
    idb, ones = aux_inputs()
    return {
        "qT": np.ascontiguousarray(q[b, sl, :].T),
        "kT": np.ascontiguousarray(k[b].T),
        "vT": np.ascontiguousarray(v[b].T),
        "mT": np.ascontiguousarray(mask[b, sl, :].T.astype(np.float32)),
        "WqT": WqT,
        "WkT": WkT,
        "WvT": WvT,
        "WoT": WoT,
        "gamma": gamma,
        "beta": beta,
        "idb": idb,
        "ones": ones,
    }


def kernel(q, k, v, mask, Wq, Wk, Wv, Wo, gamma, beta):
    q = np.asarray(q, np.float32)
    k = np.asarray(k, np.float32)
    v = np.asarray(v, np.float32)
    mask = np.asarray(mask)
    WqT = np.ascontiguousarray(np.asarray(Wq, np.float32).T)
    WkT = np.ascontiguousarray(np.asarray(Wk, np.float32).T)
    WvT = np.ascontiguousarray(np.asarray(Wv, np.float32).T)
    WoT = np.ascontiguousarray(np.asarray(Wo, np.float32).T)
    gamma = np.ascontiguousarray(np.asarray(gamma, np.float32))
    beta = np.ascontiguousarray(np.asarray(beta, np.float32))

    nc = build_nc()
    in_maps = [
        _core_inputs(q, k, v, mask, WqT, WkT, WvT, WoT, gamma, beta, c)
        for c in range(NCORES)
    ]
    res = run_bass_kernel_spmd(nc, in_maps, list(range(NCORES)))

    attn = np.empty((B, H, S, S), np.float32)
    out = np.empty((B, S, DMODEL), np.float32)
    for c in range(NCORES):
        b, hf = divmod(c, 2)
        sl = slice(hf * SQ, (hf + 1) * SQ)
        r = res.results[c]
        attn[b, :, sl, :] = np.asarray(r["attnT"]).transpose(0, 2, 1)
        out[b, sl, :] = np.asarray(r["out"])
    return attn, out


if __name__ == "__main__":
    import reference

    inputs = {kk: np.asarray(vv) for kk, vv in reference.setup_inputs().items()}
    a, o = kernel(**inputs)
    print(a.shape, o.shape)


# revision 15
# speedup vs baseline: 1.1299x; 1.1299x over previous
"""Trainium2 Bass kernel: multi-head attention (B=4, S=1024, H=16, dk=dv=64, D=1024)
returning (attn [B,H,S,S], out [B,S,D]) like the reference nn.Module.

Sharding: 8 cores = 4 batches x 2 query-halves (pure data parallel, no collectives).
Each core handles (batch b, 512 query rows) against all 1024 keys:
  - Q/K projections + scores on the PE in float32r (the attention output needs
    ~fp32 score precision); V path and O-projection in bf16 (only affects `out`,
    which has a residual + layernorm dampening the error).
  - scores^T per head in [k, q] layout; score matmuls for head pairs issued
    adjacently so the two K=64 matmuls run concurrently in different PE
    row-groups; exp on ScalarE with the 1/sqrt(dk) scale folded in; 0/1 mask
    applied as a multiply (masked lanes become exactly 0 like the reference).
  - ctx^T = vh_aug.T @ p with an extra ones-column on vh, so the softmax row
    sums fall out of the same matmul (PSUM row dk); 1/sum via the fast
    Newton-Raphson reciprocal; gpsimd broadcasts it across partitions and
    does the in-place attention normalize (DVE is busier).
  - O-projection in bf16, residual added on DVE, then layernorm.
Device emits attn^T [H, Sk, Sq_shard]; the host transposes back and reassembles.
"""

import sys
from contextlib import ExitStack

import numpy as np

try:
    import concourse.bass as bass
except ImportError:  # grading env fallback
    sys.path.insert(0, "/opt/trn_rl_repo")
    import concourse.bass as bass

import concourse.tile as tile
import ml_dtypes
from concourse import bacc, mybir
from concourse._compat import with_exitstack
from concourse.bass_utils import run_bass_kernel_spmd

F32 = mybir.dt.float32
F32R = mybir.dt.float32r
BF16 = mybir.dt.bfloat16
AF = mybir.ActivationFunctionType
ALU = mybir.AluOpType
P = 128

# full-problem dims
B, S, DMODEL, H, DK = 4, 1024, 1024, 16, 64
NCORES = 8
SQ = S // 2  # queries per core
LN_EPS = 1e-6


@with_exitstack
def _mha_kernel(ctx: ExitStack, tc, outs, ins, dm=DMODEL, sq=SQ, sk=S, nheads=H):
    nc = tc.nc
    dk = DK
    nt = dm // P  # model-dim 128-tiles
    kt = sk // P  # key-seq 128-tiles
    qm = sq // P  # query 128-tiles
    nf = min(512, dm)  # free-dim chunk for matmul N (PSUM bank limit)
    nhalf = max(1, dm // nf)
    assert nheads * dk == dm and sq <= 512

    qT, kT, vT, mT = ins["qT"], ins["kT"], ins["vT"], ins["mT"]
    WqT, WkT, WvT, WoT = ins["WqT"], ins["WkT"], ins["WvT"], ins["WoT"]
    gamma, beta = ins["gamma"], ins["beta"]
    attnT_o, out_o = outs["attnT"], outs["out"]

    persist = ctx.enter_context(tc.tile_pool(name="persist", bufs=1))
    qhT_sb = persist.tile([P, nt, sq], F32R, tag="qhT")  # [hd, q]
    khT_sb = persist.tile([P, nt, sk], F32R, tag="khT")  # [hd, k]
    # vh with a ones column per head: [k-part, kt, h, dv|1|pad]; col dk == 1.0
    vh_sb = persist.tile([P, kt, nheads, dk + 4], F32R, tag="vh")
    ctxT_sb = persist.tile([P, nt, sq], BF16, tag="ctxT")  # [hdv, q]
    nc.sync.dma_start(
        out=vh_sb[:, :, :, dk],
        in_=ins["ones"].rearrange("p (t h) -> p t h", h=nheads),
    )

    def w_stream(pool, w_dram, n0, width, dt):
        w_sb = pool.tile([P, nt, width], dt, tag="w")
        nc.sync.dma_start(
            out=w_sb,
            in_=w_dram.rearrange("(t p) n -> p t n", p=P)[:, :, n0 : n0 + width],
        )
        return w_sb

    with tc.tile_pool(name="psp", bufs=6, space="PSUM") as psp, tc.tile_pool(
        name="psc", bufs=2, space="PSUM"
    ) as psc:
        # ---- phase 1a: q-projection (qT resident; WqT halves streamed once)
        with (
            tc.tile_pool(name="qx", bufs=1) as qxp,
            tc.tile_pool(name="wq", bufs=2) as wqp,
        ):
            qT_sb = qxp.tile([P, nt, sq], F32R, tag="qT")
            nc.sync.dma_start(out=qT_sb, in_=qT.rearrange("(t p) s -> p t s", p=P))
            for half in range(nhalf):
                w_sb = w_stream(wqp, WqT, half * nf, nf, F32R)
                for mm in range(half * nf // P, (half * nf + nf) // P):
                    loc = mm * P - half * nf
                    pt = psp.tile([P, sq], F32, tag="ps")
                    for kk in range(nt):
                        nc.tensor.matmul(
                            pt,
                            w_sb[:, kk, loc : loc + P],
                            qT_sb[:, kk, :],
                            start=(kk == 0),
                            stop=(kk == nt - 1),
                        )
                    nc.any.tensor_copy(out=qhT_sb[:, mm, :], in_=pt)

        # ---- phase 1b: k-projection (kT resident; WkT halves streamed once)
        with (
            tc.tile_pool(name="kx", bufs=1) as kxp,
            tc.tile_pool(name="wk", bufs=2) as wkp,
        ):
            kT_sb = kxp.tile([P, nt, sk], F32R, tag="kT")
            nc.sync.dma_start(out=kT_sb, in_=kT.rearrange("(t p) s -> p t s", p=P))
            for half in range(nhalf):
                w_sb = w_stream(wkp, WkT, half * nf, nf, F32R)
                for mm in range(half * nf // P, (half * nf + nf) // P):
                    loc = mm * P - half * nf
                    for n0 in range(0, sk, nf):
                        pt = psp.tile([P, nf], F32, tag="ps")
                        for kk in range(nt):
                            nc.tensor.matmul(
                                pt,
                                w_sb[:, kk, loc : loc + P],
                                kT_sb[:, kk, n0 : n0 + nf],
                                start=(kk == 0),
                                stop=(kk == nt - 1),
                            )
                        nc.any.tensor_copy(out=khT_sb[:, mm, n0 : n0 + nf], in_=pt)

        # ---- phase 1c: v-projection in bf16 (vT resident; WvT halves once)
        with (
            tc.tile_pool(name="vx", bufs=1) as vxp,
            tc.tile_pool(name="wv", bufs=2) as wvp,
        ):
            vT_sb = vxp.tile([P, nt, sk], BF16, tag="vT")
            nc.sync.dma_start(out=vT_sb, in_=vT.rearrange("(t p) s -> p t s", p=P))
            for n0 in range(0, dm, nf):
                w_sb = w_stream(wvp, WvT, n0, nf, BF16)
                for mm in range(kt):
                    pt = psp.tile([P, nf], F32, tag="ps")
                    for kk in range(nt):
                        nc.tensor.matmul(
                            pt,
                            vT_sb[:, kk, mm * P : (mm + 1) * P],
                            w_sb[:, kk, :],
                            start=(kk == 0),
                            stop=(kk == nt - 1),
                        )
                    nc.any.tensor_copy(
                        out=vh_sb[:, mm, n0 // dk : n0 // dk + nf // dk, 0:dk],
                        in_=pt.rearrange("p (h d) -> p h d", d=dk),
                    )

        # ---- phase 2: attention, head pairs (score MMs adjacent -> concurrent
        # row-groups); wo pool opened early so the bf16 WoT halves prefetch
        wop = ctx.enter_context(tc.tile_pool(name="wo", bufs=2))
        with (
            tc.tile_pool(name="att", bufs=3) as att,
            tc.tile_pool(name="attm", bufs=1) as attm,
            tc.tile_pool(name="atts", bufs=3) as atts,
        ):
            m_sb = attm.tile([P, kt, sq], F32, tag="m")  # mask^T as 0.0/1.0
            nc.sync.dma_start(out=m_sb, in_=mT.rearrange("(t p) s -> p t s", p=P))

            for jp in range(nheads // 2):
                pab = [att.tile([P, kt, sq], F32R, tag="p", name=f"p{i}") for i in range(2)]
                for t in range(kt):
                    sts = [psp.tile([P, sq], F32, tag="ps", name=f"st{i}") for i in range(2)]
                    for i in range(2):
                        r0 = i * dk
                        nc.tensor.matmul(
                            sts[i],
                            khT_sb[r0 : r0 + dk, jp, t * P : (t + 1) * P],
                            qhT_sb[r0 : r0 + dk, jp, :],
                            start=True,
                            stop=True,
                        )
                    for i in range(2):
                        nc.scalar.activation(
                            out=pab[i][:, t, :], in_=sts[i], func=AF.Exp,
                            scale=1.0 / float(np.sqrt(dk)),
                        )
                        nc.vector.tensor_mul(
                            pab[i][:, t, :], pab[i][:, t, :], m_sb[:, t, :]
                        )
                for i in range(2):
                    hh, pp = 2 * jp + i, pab[i]
                    ct = psc.tile([P, sq], F32, tag="ps_c")
                    for t in range(kt):
                        nc.tensor.matmul(
                            ct[0 : dk + 1, :],
                            vh_sb[:, t, hh, 0 : dk + 1],
                            pp[:, t, :],
                            start=(t == 0),
                            stop=(t == kt - 1),
                        )
                    rec = atts.tile([1, sq], F32, tag="rec")
                    nc.vector.reciprocal(out=rec, in_=ct[dk : dk + 1, :])
                    rb = atts.tile([P, sq], F32, tag="rb")
                    nc.gpsimd.partition_broadcast(rb, rec[0:1, :])
                    # normalized ctx^T straight into the O-projection layout
                    nc.vector.tensor_mul(
                        ctxT_sb[i * dk : (i + 1) * dk, jp, :], ct[0:dk, :], rb[0:dk, :]
                    )
                    # normalize attention in place (gpsimd; f32 view of f32r bits)
                    rb_b = bass.AP(
                        tensor=rb.tensor, offset=rb.offset,
                        ap=[rb.ap[0], [0, kt], rb.ap[1]],
                    )
                    nc.vector.tensor_mul(pp, pp, rb_b)
                    nc.sync.dma_start(
                        out=attnT_o.rearrange("h (t p) q -> h p t q", p=P)[hh], in_=pp
                    )

    # ---- phase 3: O-projection (bf16) + residual + layernorm
    with (
        tc.tile_pool(name="pso", bufs=qm, space="PSUM") as pso,
        tc.tile_pool(name="ln", bufs=2) as lnp,
        tc.tile_pool(name="lns", bufs=3) as lns,
        tc.tile_pool(name="singles", bufs=1) as singles,
    ):
        qres_sb = singles.tile([P, qm, dm], F32, tag="qres")
        nc.sync.dma_start(
            out=qres_sb, in_=ins["q_res"].rearrange("(m p) d -> p m d", p=P)
        )
        gb = singles.tile([P, dm], F32, tag="gb")
        nc.sync.dma_start(
            out=gb,
            in_=bass.AP(tensor=gamma.tensor, offset=gamma.offset, ap=[[0, P], [1, dm]]),
        )
        bb = singles.tile([P, dm], F32, tag="bb")
        nc.sync.dma_start(
            out=bb,
            in_=bass.AP(tensor=beta.tensor, offset=beta.offset, ap=[[0, P], [1, dm]]),
        )
        epst = singles.tile([P, 1], F32, tag="eps")
        nc.vector.memset(epst, LN_EPS)

        xts = [pso.tile([P, dm], F32, tag="ps_o", name=f"xt{i}") for i in range(qm)]
        for n0 in range(0, dm, nf):
            w_sb = wop.tile([P, nt, nf], BF16, tag="wo")
            nc.sync.dma_start(
                out=w_sb,
                in_=WoT.rearrange("(t p) n -> p t n", p=P)[:, :, n0 : n0 + nf],
            )
            for mm in range(qm):
                for j in range(nt):
                    nc.tensor.matmul(
                        xts[mm][:, n0 : n0 + nf],
                        ctxT_sb[:, j, mm * P : (mm + 1) * P],
                        w_sb[:, j, :],
                        start=(j == 0),
                        stop=(j == nt - 1),
                    )
        for mm in range(qm):
            y0 = lnp.tile([P, dm], F32, tag="y0")
            nc.vector.tensor_add(y0, xts[mm], qres_sb[:, mm, :])  # residual
            nsub = max(1, dm // 512)
            stats = lns.tile([P, nsub, 6], F32, tag="stats")
            for sg in range(nsub):
                w = dm // nsub
                nc.vector.bn_stats(out=stats[:, sg, :], in_=y0[:, sg * w : (sg + 1) * w])
            mv = lns.tile([P, 2], F32, tag="mv")
            nc.vector.bn_aggr(out=mv, in_=stats)
            rstd = lns.tile([P, 1], F32, tag="rstd")
            nc.scalar.activation(out=rstd, in_=mv[:, 1:2], func=AF.Sqrt, bias=epst)
            rstd2 = lns.tile([P, 1], F32, tag="rstd2")
            nc.vector.reciprocal(out=rstd2, in_=rstd)
            nmu = lns.tile([P, 1], F32, tag="nmu")
            nc.vector.tensor_scalar(
                out=nmu, in0=mv[:, 0:1], scalar1=-1.0, scalar2=None, op0=ALU.mult
            )
            yt = lnp.tile([P, dm], F32, tag="y")
            nc.vector.tensor_scalar(
                out=yt, in0=y0, scalar1=nmu, scalar2=rstd2, op0=ALU.add, op1=ALU.mult
            )
            ot = lnp.tile([P, dm], F32, tag="ot")
            nc.vector.tensor_mul(ot, yt, gb)
            nc.vector.tensor_add(ot, ot, bb)
            nc.sync.dma_start(out=out_o.rearrange("(m p) d -> m p d", p=P)[mm], in_=ot)


_CACHE = {}


def build_nc(dm=DMODEL, sq=SQ, sk=S, nheads=H):
    key = (dm, sq, sk, nheads)
    if key in _CACHE:
        return _CACHE[key]
    nc = bacc.Bacc("TRN2", target_bir_lowering=False, debug=False, num_devices=NCORES)
    ins = {}

    def din(name, shape, dt=F32):
        ins[name] = nc.dram_tensor(name, list(shape), dt, kind="ExternalInput").ap()

    din("qT", (dm, sq), F32R)
    din("kT", (dm, sk), F32R)
    din("vT", (dm, sk), BF16)
    din("mT", (sk, sq))
    din("q_res", (sq, dm))
    din("WqT", (dm, nheads * DK), F32R)
    din("WkT", (dm, nheads * DK), F32R)
    din("WvT", (dm, nheads * DK), BF16)
    din("WoT", (nheads * DK, dm), BF16)
    din("gamma", (dm,))
    din("beta", (dm,))
    din("ones", (P, (sk // P) * nheads), F32R)
    outs = {
        "attnT": nc.dram_tensor("attnT", [nheads, sk, sq], F32R, kind="ExternalOutput").ap(),
        "out": nc.dram_tensor("out", [sq, dm], F32, kind="ExternalOutput").ap(),
    }
    with tile.TileContext(nc) as tc:
        _mha_kernel(tc, outs, ins, dm=dm, sq=sq, sk=sk, nheads=nheads)
    nc.compile()
    _CACHE[key] = nc
    return nc


def _core_inputs(q, k, v, mask, WqT, WkT, WvT, WoT, gamma, beta, c):
    b, hf = divmod(c, 2)
    sl = slice(hf * SQ, (hf + 1) * SQ)
    return {
        "qT": np.ascontiguousarray(q[b, sl, :].T),
        "kT": np.ascontiguousarray(k[b].T),
        "vT": np.ascontiguousarray(v[b].T).astype(ml_dtypes.bfloat16),
        "mT": np.ascontiguousarray(mask[b, sl, :].T.astype(np.float32)),
        "q_res": np.ascontiguousarray(q[b, sl, :]),
        "WqT": WqT,
        "WkT": WkT,
        "WvT": WvT,
        "WoT": WoT,
        "gamma": gamma,
        "beta": beta,
        "ones": np.ones((P, (S // P) * H), np.float32),
    }


def kernel(q, k, v, mask, Wq, Wk, Wv, Wo, gamma, beta):
    q = np.asarray(q, np.float32)
    k = np.asarray(k, np.float32)
    v = np.asarray(v, np.float32)
    mask = np.asarray(mask)
    WqT = np.ascontiguousarray(np.asarray(Wq, np.float32).T)
    WkT = np.ascontiguousarray(np.asarray(Wk, np.float32).T)
    WvT = np.ascontiguousarray(np.asarray(Wv, np.float32).T).astype(ml_dtypes.bfloat16)
    WoT = np.ascontiguousarray(np.asarray(Wo, np.float32).T).astype(ml_dtypes.bfloat16)
    gamma = np.ascontiguousarray(np.asarray(gamma, np.float32))
    beta = np.ascontiguousarray(np.asarray(beta, np.float32))

    nc = build_nc()
    in_maps = [
        _core_inputs(q, k, v, mask, WqT, WkT, WvT, WoT, gamma, beta, c)
        for c in range(NCORES)
    ]
    res = run_bass_kernel_spmd(nc, in_maps, list(range(NCORES)))

    attn = np.empty((B, H, S, S), np.float32)
    out = np.empty((B, S, DMODEL), np.float32)
    for c in range(NCORES):
        b, hf = divmod(c, 2)
        sl = slice(hf * SQ, (hf + 1) * SQ)
        r = res.results[c]
        attn[b, :, sl, :] = np.asarray(r["attnT"]).transpose(0, 2, 1)
        out[b, sl, :] = np.asarray(r["out"])
    return attn, out


if __name__ == "__main__":
    import reference

    inputs = {kk: np.asarray(vv) for kk, vv in reference.setup_inputs().items()}
    a, o = kernel(**inputs)
    print(a.shape, o.shape)
